# revision 4
# baseline (speedup 1.0000x reference)
"""Trainium2 Bass kernel for a 2-layer LSTM (B=4096, T=168, D=16, H=96) + FC head.

Strategy: pure data parallel over 8 NeuronCores (512 batch rows each).
Per core, gate-major layout: the recurrent matmul computes gates.T
[gate, batch] with weights stationary on the PE, so hidden state h stays in
[feature, batch] layout across steps and never needs a transpose.

v2 changes vs v1 (1493us):
- bf16 matmuls (lhsT + rhs): 1 cycle/row at any N, halves weight-load time,
  halves DMA, and lets DVE run its 2x/4x perf modes on the elementwise ops.
- ONE sigmoid ACT op per layer-step covering all 4 gates [128, 2048]: gate g's
  weights+bias are pre-scaled by 2 so sigmoid(2g) comes out, and
  tanh(g) = 2*sigmoid(2g) - 1 is fixed up with a 4x-mode DVE tensor_scalar.
  This cuts ACT from 3 to 2 instructions per layer-step (ACT is the
  bottleneck engine: cost = free-size * 0.83ns, dtype-independent).
- tanh(c) stays a real ACT tanh (same instruction count either way).
- All elementwise tiles bf16 in SBUF: tensor_tensor runs 2x (2x_1p),
  tensor_scalar runs 4x (4x_2p).

Gates are ordered [i, f, o, g], each padded to 128 PSUM partitions. Biases
ride along in the matmul via a constant-1.0 input row.
"""

import numpy as np
import ml_dtypes

import concourse.bass as bass
import concourse.bacc as bacc
import concourse.tile as tile
from concourse import mybir
from concourse.bass_utils import run_bass_kernel_spmd

B, T, D, H = 4096, 168, 16, 96
NCORES = 8
BS = B // NCORES  # 512 batch rows per core
F32 = mybir.dt.float32
BF16 = mybir.dt.bfloat16
SIG = mybir.ActivationFunctionType.Sigmoid
TANH = mybir.ActivationFunctionType.Tanh
MULT = mybir.AluOpType.mult
SUB = mybir.AluOpType.subtract

# gate row slices in torch order (i, f, g, o) -> our tile order [i, f, o, g]
_GATE_SLICES = [(0, 96), (96, 192), (288, 384), (192, 288)]

TRACE = False
LAST = {}
T_RUN = T
MM_DT = BF16
# PE idle-filler matmuls per iteration: keeps the PE streaming continuously so
# it ramps to (and holds) its 2.4GHz p-state instead of the 1.2GHz mid state
# it falls back to after any idle gap. Filler output is garbage written to a
# PSUM region that the next real start=True matmul fully overwrites.
JUNK_MM = 6
JUNK_N = 512


def _prep_weights(Wih0, Whh0, bih0, bhh0, Wih1, Whh1, bih1, bhh1, Wfc, bfc):
    w0 = np.zeros((113, 512), np.float32)  # rows: h(96), x(16), const(1)
    w1a = np.zeros((96, 512), np.float32)  # rows: h1(96)
    w1b = np.zeros((97, 512), np.float32)  # rows: h2(96), const(1)
    for gi, (r0, r1) in enumerate(_GATE_SLICES):
        c0, c1 = 128 * gi, 128 * gi + 96
        # gate 3 is g: pre-scale by 2 so the merged sigmoid computes
        # sigmoid(2g) and tanh(g) = 2*sigmoid(2g) - 1
        sc = 2.0 if gi == 3 else 1.0
        w0[0:96, c0:c1] = sc * Whh0[r0:r1, :].T
        w0[96:112, c0:c1] = sc * Wih0[r0:r1, :].T
        w0[112, c0:c1] = sc * (bih0[r0:r1] + bhh0[r0:r1])
        w1a[:, c0:c1] = sc * Wih1[r0:r1, :].T
        w1b[0:96, c0:c1] = sc * Whh1[r0:r1, :].T
        w1b[96, c0:c1] = sc * (bih1[r0:r1] + bhh1[r0:r1])
    wfc = np.zeros((97, 1), np.float32)
    wfc[0:96, 0] = Wfc[0, :]
    wfc[96, 0] = bfc[0]
    bf = ml_dtypes.bfloat16
    return w0.astype(bf), w1a.astype(bf), w1b.astype(bf), wfc.astype(bf)


def _build_nc():
    nc = bacc.Bacc("TRN2", target_bir_lowering=False)
    xs_d = nc.dram_tensor("xs", [T, D + 1, BS], MM_DT, kind="ExternalInput")
    w0_d = nc.dram_tensor("w0", [113, 512], MM_DT, kind="ExternalInput")
    w1a_d = nc.dram_tensor("w1a", [96, 512], MM_DT, kind="ExternalInput")
    w1b_d = nc.dram_tensor("w1b", [97, 512], MM_DT, kind="ExternalInput")
    wfc_d = nc.dram_tensor("wfc", [97, 1], MM_DT, kind="ExternalInput")
    y_d = nc.dram_tensor("y", [1, BS], F32, kind="ExternalOutput")

    with tile.TileContext(nc) as tc:
        with (
            tc.tile_pool(name="persist", bufs=1) as P,
            tc.tile_pool(name="sig", bufs=2) as SIGP,
            tc.tile_pool(name="tgp", bufs=2) as TGP,
            tc.tile_pool(name="tcp", bufs=2) as TCP,
            tc.tile_pool(name="qp", bufs=2) as QPP,
            tc.tile_pool(name="ps", bufs=1, space="PSUM") as PSP,
        ):
            # DMA into staging tiles, then DVE-copy into the tiles matmuls
            # read, so matmul waits only involve {DVE, ACT} sems.
            w0_g = P.tile([113, 512], MM_DT, tag="w0_g")
            w1a_g = P.tile([96, 512], MM_DT, tag="w1a_g")
            w1b_g = P.tile([97, 512], MM_DT, tag="w1b_g")
            wfc_g = P.tile([97, 1], MM_DT, tag="wfc_g")
            nc.gpsimd.dma_start(out=w0_g[:, :], in_=w0_d[:, :])
            nc.gpsimd.dma_start(out=w1a_g[:, :], in_=w1a_d[:, :])
            nc.gpsimd.dma_start(out=w1b_g[:, :], in_=w1b_d[:, :])
            nc.gpsimd.dma_start(out=wfc_g[:, :], in_=wfc_d[:, :])
            w0_s = P.tile([113, 512], MM_DT, tag="w0")
            w1a_s = P.tile([96, 512], MM_DT, tag="w1a")
            w1b_s = P.tile([97, 512], MM_DT, tag="w1b")
            wfc_s = P.tile([97, 1], MM_DT, tag="wfc")
            nc.vector.tensor_copy(w0_s[:, :], w0_g[:, :])
            nc.vector.tensor_copy(w1a_s[:, :], w1a_g[:, :])
            nc.vector.tensor_copy(w1b_s[:, :], w1b_g[:, :])
            nc.vector.tensor_copy(wfc_s[:, :], wfc_g[:, :])

            # rhs0: [h0(0:96); x_t(96:112); 1.0(112)]  rhs1: [h2(0:96); 1.0(96)]
            rhs0 = [P.tile([113, BS], MM_DT, tag=f"rhs0_{i}", name=f"rhs0_{i}") for i in range(2)]
            rhs1 = [P.tile([97, BS], MM_DT, tag=f"rhs1_{i}", name=f"rhs1_{i}") for i in range(2)]
            c0 = P.tile([96, BS], BF16, tag="c0")
            c1 = P.tile([96, BS], BF16, tag="c1")
            for i in range(2):
                nc.vector.memset(rhs0[i][:, :], 0.0)
                nc.vector.memset(rhs1[i][:, :], 0.0)
                nc.vector.memset(rhs1[i][96:97, :], 1.0)
            nc.vector.memset(c0[:, :], 0.0)
            nc.vector.memset(c1[:, :], 0.0)

            nc.gpsimd.dma_start(out=rhs0[0][96:113, :], in_=xs_d[0, :, :])

            def l0_block(t):
                # layer-0 step t: consumes rhs0[t%2], writes h1_t into
                # rhs0[(t+1)%2] rows 0:96
                cur, nxt = t % 2, (t + 1) % 2
                if t + 1 < T_RUN:
                    nc.gpsimd.dma_start(
                        out=rhs0[nxt][96:113, :], in_=xs_d[t + 1, :, :]
                    )
                g0 = PSP.tile([128, 2048], F32, tag="g0", name="g0")
                for g in range(4):
                    nc.tensor.matmul(
                        out=g0[:, 512 * g : 512 * (g + 1)],
                        lhsT=w0_s[:, 128 * g : 128 * (g + 1)],
                        rhs=rhs0[cur][:, :],
                        start=True,
                        stop=True,
                    )
                # one sigmoid over all 4 gates: [i, f, o, 2g]
                s0 = SIGP.tile([128, 2048], BF16, tag="sig0", name="sig0")
                nc.scalar.activation(out=s0[:, :], in_=g0[:, :], func=SIG)
                # tanh(g) = 2*sigmoid(2g) - 1  (4x-mode tensor_scalar)
                tg0 = TGP.tile([96, BS], BF16, tag="tg0", name="tg0")
                nc.vector.tensor_scalar(
                    tg0[:, :], s0[0:96, 1536:2048], 2.0, 1.0, MULT, SUB
                )
                q0 = QPP.tile([96, BS], BF16, tag="q0", name="q0")
                p0 = QPP.tile([96, BS], BF16, tag="p0", name="p0")
                nc.vector.tensor_mul(q0[:, :], s0[0:96, 512:1024], c0[:, :])
                nc.vector.tensor_mul(p0[:, :], s0[0:96, 0:512], tg0[:, :])
                nc.vector.tensor_add(c0[:, :], q0[:, :], p0[:, :])
                tc0 = TCP.tile([96, BS], BF16, tag="tc0", name="tc0")
                nc.scalar.activation(out=tc0[:, :], in_=c0[:, :], func=TANH)
                nc.vector.tensor_mul(
                    rhs0[nxt][0:96, :], s0[0:96, 1024:1536], tc0[:, :]
                )

            def l1_mms_pre(t):
                # Early L1 work for step t: PE idle-filler matmuls (output
                # garbage, fully overwritten by the start=True w1b matmuls
                # below) plus the w1b*[h2;1] accumulation, which only needs
                # h2(t-1) and so can run while the PE would otherwise idle
                # waiting for h1(t).
                cur = t % 2
                g1 = PSP.tile([128, 2048], F32, tag="g1", name="g1")
                for j in range(JUNK_MM):
                    nc.tensor.matmul(
                        out=g1[:, 0:JUNK_N],
                        lhsT=w1b_s[:, 0:128],
                        rhs=rhs1[cur][0:97, 0:JUNK_N],
                        start=True,
                        stop=(j == JUNK_MM - 1),
                        skip_group_check=True,
                    )
                for g in range(4):
                    nc.tensor.matmul(
                        out=g1[:, 512 * g : 512 * (g + 1)],
                        lhsT=w1b_s[:, 128 * g : 128 * (g + 1)],
                        rhs=rhs1[cur][0:97, :],
                        start=True,
                        stop=False,
                    )
                return g1

            def l1_rest(t, g1):
                # layer-1 step t: w1a*h1(t) accumulation, activations, cell
                # update; writes h2_t into rhs1[(t+1)%2]
                nxt = (t + 1) % 2
                for g in range(4):
                    nc.tensor.matmul(
                        out=g1[:, 512 * g : 512 * (g + 1)],
                        lhsT=w1a_s[:, 128 * g : 128 * (g + 1)],
                        rhs=rhs0[nxt][0:96, :],
                        start=False,
                        stop=True,
                    )
                s1 = SIGP.tile([128, 2048], BF16, tag="sig1", name="sig1")
                nc.scalar.activation(out=s1[:, :], in_=g1[:, :], func=SIG)
                tg1 = TGP.tile([96, BS], BF16, tag="tg1", name="tg1")
                nc.vector.tensor_scalar(
                    tg1[:, :], s1[0:96, 1536:2048], 2.0, 1.0, MULT, SUB
                )
                q1 = QPP.tile([96, BS], BF16, tag="q1", name="q1")
                p1 = QPP.tile([96, BS], BF16, tag="p1", name="p1")
                nc.vector.tensor_mul(q1[:, :], s1[0:96, 512:1024], c1[:, :])
                nc.vector.tensor_mul(p1[:, :], s1[0:96, 0:512], tg1[:, :])
                nc.vector.tensor_add(c1[:, :], q1[:, :], p1[:, :])
                tc1 = TCP.tile([96, BS], BF16, tag="tc1", name="tc1")
                nc.scalar.activation(out=tc1[:, :], in_=c1[:, :], func=TANH)
                nc.vector.tensor_mul(
                    rhs1[nxt][0:96, :], s1[0:96, 1024:1536], tc1[:, :]
                )

            # Software-pipelined emission. Per iteration t the per-engine
            # queue order is:
            #   PE:  junk+w1b(t) [ready early] -> L0 mms(t+1) -> w1a(t)
            #   ACT: sig0(t+1) -> tanh0(t+1) -> sig1(t) -> tanh1(t)
            #   DVE: L0 chain(t+1) -> h1-mul(t+1) -> L1 chain(t) -> h2-mul(t)
            # The 4-deep per-engine wait queues let a ready instruction pass
            # a parked one, so this order prioritizes the recurrence-critical
            # L0 spine while L1 work fills the gaps.
            l0_block(0)
            for t in range(T_RUN):
                g1 = l1_mms_pre(t)
                if t + 1 < T_RUN:
                    l0_block(t + 1)
                l1_rest(t, g1)

            # ---- FC head on h2 at t = T-1 ----
            fc_ps = PSP.tile([1, 512], F32, tag="g0")
            nc.tensor.matmul(
                out=fc_ps[:, :],
                lhsT=wfc_s[:, :],
                rhs=rhs1[T_RUN % 2][0:97, :],
                start=True,
                stop=True,
            )
            y_s = P.tile([1, 512], F32, tag="y")
            nc.vector.tensor_copy(y_s[:, :], fc_ps[:, :])
            nc.gpsimd.dma_start(out=y_d[:, :], in_=y_s[:, :])
    nc.compile()
    return nc



def _ensure_ntff_hook():
    """Provide antenv.axon_hooks (absent in this image) so trace=True works."""
    import sys, types, ctypes, contextlib
    try:
        import antenv.axon_hooks  # noqa: F401
        return
    except ImportError:
        pass
    mod = types.ModuleType("antenv.axon_hooks")
    holder = {}
    mod.set_axon_ntff_profile_hook = lambda h: holder.__setitem__("h", h)
    mod.get_axon_ntff_profile_hook = lambda: holder.get("h")
    sys.modules["antenv.axon_hooks"] = mod
    lib = ctypes.CDLL("/opt/axon/libaxon_pjrt.so")
    if not hasattr(lib, "axon_start_nrt_profile"):
        return
    lib.axon_start_nrt_profile.argtypes = [
        ctypes.POINTER(ctypes.c_int64), ctypes.c_size_t]
    lib.axon_start_nrt_profile.restype = ctypes.c_int64
    lib.axon_stop_nrt_profile.argtypes = [ctypes.c_char_p]
    lib.axon_stop_nrt_profile.restype = ctypes.c_int64

    @contextlib.contextmanager
    def _hook(output_dir, device_ids):
        import jax
        jax.devices()
        if device_ids:
            ids = (ctypes.c_int64 * len(device_ids))(*device_ids)
            rc = lib.axon_start_nrt_profile(ids, len(device_ids))
        else:
            rc = lib.axon_start_nrt_profile(None, 0)
        if rc != 0:
            raise RuntimeError(f"axon_start_nrt_profile rc={rc}")
        try:
            yield
        finally:
            n = lib.axon_stop_nrt_profile(str(output_dir).encode())
            print(f"ntff profile: {n} file(s) written to {output_dir}")

    mod.set_axon_ntff_profile_hook(_hook)


def _patch_upload():
    """Skip artifact upload to remote storage (no share in this container)."""
    import concourse.bass_utils as bu
    bu.upload_artifacts = lambda tmpdir: tmpdir


_NC = None


def kernel(x, Wih0, Whh0, bih0, bhh0, Wih1, Whh1, bih1, bhh1, Wfc, bfc):
    global _NC
    arrs = [np.asarray(a, np.float32) for a in (
        x, Wih0, Whh0, bih0, bhh0, Wih1, Whh1, bih1, bhh1, Wfc, bfc)]
    x = arrs[0]
    w0, w1a, w1b, wfc = _prep_weights(*arrs[1:])
    if _NC is None:
        _NC = _build_nc()
    bf = ml_dtypes.bfloat16
    in_maps = []
    for core in range(NCORES):
        xt = x[core * BS : (core + 1) * BS].transpose(1, 2, 0)  # [T, D, BS]
        xs = np.concatenate(
            [xt, np.ones((T, 1, BS), np.float32)], axis=1
        ).astype(bf)  # [T, D+1, BS] with const-1 row
        in_maps.append({"xs": xs, "w0": w0, "w1a": w1a, "w1b": w1b, "wfc": wfc})
    if TRACE:
        _ensure_ntff_hook()
        _patch_upload()
    import tempfile
    tdir = tempfile.mkdtemp(prefix="lstm_prof_") if TRACE else None
    res = run_bass_kernel_spmd(
        _NC, in_maps, core_ids=list(range(NCORES)), trace=TRACE, tmpdir=tdir
    )
    LAST["tmpdir"] = tdir
    LAST["exec_time_ns"] = res.exec_time_ns
    LAST["profile_json"] = res.profile_json
    y = np.concatenate([res.results[i]["y"][0] for i in range(NCORES)])
    return y.astype(np.float32)


# revision 6
# speedup vs baseline: 1.2569x; 1.2569x over previous
"""Trainium2 Bass kernel for a 2-layer LSTM (B=4096, T=168, D=16, H=96) + FC head.

Strategy: pure data parallel over 8 NeuronCores (512 batch rows each).
Per core, gate-major layout: the recurrent matmul computes gates.T
[gate, batch] with weights stationary on the PE, so hidden state h stays in
[feature, batch] layout across steps and never needs a transpose.

v2 changes vs v1 (1493us):
- bf16 matmuls (lhsT + rhs): 1 cycle/row at any N, halves weight-load time,
  halves DMA, and lets DVE run its 2x/4x perf modes on the elementwise ops.
- ONE sigmoid ACT op per layer-step covering all 4 gates [128, 2048]: gate g's
  weights+bias are pre-scaled by 2 so sigmoid(2g) comes out, and
  tanh(g) = 2*sigmoid(2g) - 1 is fixed up with a 4x-mode DVE tensor_scalar.
  This cuts ACT from 3 to 2 instructions per layer-step (ACT is the
  bottleneck engine: cost = free-size * 0.83ns, dtype-independent).
- tanh(c) stays a real ACT tanh (same instruction count either way).
- All elementwise tiles bf16 in SBUF: tensor_tensor runs 2x (2x_1p),
  tensor_scalar runs 4x (4x_2p).

Gates are ordered [i, f, o, g], each padded to 128 PSUM partitions. Biases
ride along in the matmul via a constant-1.0 input row.
"""

import numpy as np
import ml_dtypes

import concourse.bass as bass
import concourse.bacc as bacc
import concourse.tile as tile
from concourse import mybir
from concourse.bass_utils import run_bass_kernel_spmd

B, T, D, H = 4096, 168, 16, 96
NCORES = 8
BS = B // NCORES  # 512 batch rows per core
F32 = mybir.dt.float32
BF16 = mybir.dt.bfloat16
SIG = mybir.ActivationFunctionType.Sigmoid
TANH = mybir.ActivationFunctionType.Tanh
MULT = mybir.AluOpType.mult
SUB = mybir.AluOpType.subtract

# gate row slices in torch order (i, f, g, o) -> our tile order [i, f, o, g]
_GATE_SLICES = [(0, 96), (96, 192), (288, 384), (192, 288)]

TRACE = False
LAST = {}
T_RUN = T
MM_DT = BF16
# PE idle-filler matmuls per iteration: keeps the PE streaming continuously so
# it ramps to (and holds) its 2.4GHz p-state instead of the 1.2GHz mid state
# it falls back to after any idle gap. Filler output is garbage written to a
# PSUM region that the next real start=True matmul fully overwrites.
JUNK_MM = 6
JUNK_N = 512


def _prep_weights(Wih0, Whh0, bih0, bhh0, Wih1, Whh1, bih1, bhh1, Wfc, bfc):
    w0 = np.zeros((113, 512), np.float32)  # rows: h(96), x(16), const(1)
    w1a = np.zeros((96, 512), np.float32)  # rows: h1(96)
    w1b = np.zeros((97, 512), np.float32)  # rows: h2(96), const(1)
    for gi, (r0, r1) in enumerate(_GATE_SLICES):
        c0, c1 = 128 * gi, 128 * gi + 96
        # gate 3 is g: pre-scale by 2 so the merged sigmoid computes
        # sigmoid(2g) and tanh(g) = 2*sigmoid(2g) - 1
        sc = 2.0 if gi == 3 else 1.0
        w0[0:96, c0:c1] = sc * Whh0[r0:r1, :].T
        w0[96:112, c0:c1] = sc * Wih0[r0:r1, :].T
        w0[112, c0:c1] = sc * (bih0[r0:r1] + bhh0[r0:r1])
        w1a[:, c0:c1] = sc * Wih1[r0:r1, :].T
        w1b[0:96, c0:c1] = sc * Whh1[r0:r1, :].T
        w1b[96, c0:c1] = sc * (bih1[r0:r1] + bhh1[r0:r1])
    wfc = np.zeros((97, 1), np.float32)
    wfc[0:96, 0] = Wfc[0, :]
    wfc[96, 0] = bfc[0]
    bf = ml_dtypes.bfloat16
    return w0.astype(bf), w1a.astype(bf), w1b.astype(bf), wfc.astype(bf)


def _build_nc():
    nc = bacc.Bacc("TRN2", target_bir_lowering=False)
    xs_d = nc.dram_tensor("xs", [T, D + 1, BS], MM_DT, kind="ExternalInput")
    w0_d = nc.dram_tensor("w0", [113, 512], MM_DT, kind="ExternalInput")
    w1a_d = nc.dram_tensor("w1a", [96, 512], MM_DT, kind="ExternalInput")
    w1b_d = nc.dram_tensor("w1b", [97, 512], MM_DT, kind="ExternalInput")
    wfc_d = nc.dram_tensor("wfc", [97, 1], MM_DT, kind="ExternalInput")
    y_d = nc.dram_tensor("y", [1, BS], F32, kind="ExternalOutput")

    with tile.TileContext(nc) as tc:
        with (
            tc.tile_pool(name="persist", bufs=1) as P,
            tc.tile_pool(name="sig", bufs=2) as SIGP,
            tc.tile_pool(name="tgp", bufs=2) as TGP,
            tc.tile_pool(name="tcp", bufs=2) as TCP,
            tc.tile_pool(name="qp", bufs=2) as QPP,
            tc.tile_pool(name="ps", bufs=1, space="PSUM") as PSP,
        ):
            # DMA into staging tiles, then DVE-copy into the tiles matmuls
            # read, so matmul waits only involve {DVE, ACT} sems.
            w0_g = P.tile([113, 512], MM_DT, tag="w0_g")
            w1a_g = P.tile([96, 512], MM_DT, tag="w1a_g")
            w1b_g = P.tile([97, 512], MM_DT, tag="w1b_g")
            wfc_g = P.tile([97, 1], MM_DT, tag="wfc_g")
            nc.gpsimd.dma_start(out=w0_g[:, :], in_=w0_d[:, :])
            nc.gpsimd.dma_start(out=w1a_g[:, :], in_=w1a_d[:, :])
            nc.gpsimd.dma_start(out=w1b_g[:, :], in_=w1b_d[:, :])
            nc.gpsimd.dma_start(out=wfc_g[:, :], in_=wfc_d[:, :])
            w0_s = P.tile([113, 512], MM_DT, tag="w0")
            w1a_s = P.tile([96, 512], MM_DT, tag="w1a")
            w1b_s = P.tile([97, 512], MM_DT, tag="w1b")
            wfc_s = P.tile([97, 1], MM_DT, tag="wfc")
            nc.vector.tensor_copy(w0_s[:, :], w0_g[:, :])
            nc.vector.tensor_copy(w1a_s[:, :], w1a_g[:, :])
            nc.vector.tensor_copy(w1b_s[:, :], w1b_g[:, :])
            nc.vector.tensor_copy(wfc_s[:, :], wfc_g[:, :])

            # rhs0: [h0(0:96); x_t(96:112); 1.0(112)]  rhs1: [h2(0:96); 1.0(96)]
            rhs0 = [P.tile([113, BS], MM_DT, tag=f"rhs0_{i}", name=f"rhs0_{i}") for i in range(2)]
            rhs1 = [P.tile([97, BS], MM_DT, tag=f"rhs1_{i}", name=f"rhs1_{i}") for i in range(2)]
            c0 = P.tile([96, BS], BF16, tag="c0")
            c1 = P.tile([96, BS], BF16, tag="c1")
            for i in range(2):
                nc.vector.memset(rhs0[i][:, :], 0.0)
                nc.vector.memset(rhs1[i][:, :], 0.0)
                nc.vector.memset(rhs1[i][96:97, :], 1.0)
            nc.vector.memset(c0[:, :], 0.0)
            nc.vector.memset(c1[:, :], 0.0)

            nc.gpsimd.dma_start(out=rhs0[0][96:113, :], in_=xs_d[0, :, :])

            def l0_block(t):
                # layer-0 step t: consumes rhs0[t%2], writes h1_t into
                # rhs0[(t+1)%2] rows 0:96
                cur, nxt = t % 2, (t + 1) % 2
                if t + 1 < T_RUN:
                    nc.gpsimd.dma_start(
                        out=rhs0[nxt][96:113, :], in_=xs_d[t + 1, :, :]
                    )
                g0 = PSP.tile([128, 2048], F32, tag="g0", name="g0")
                for g in range(4):
                    nc.tensor.matmul(
                        out=g0[:, 512 * g : 512 * (g + 1)],
                        lhsT=w0_s[:, 128 * g : 128 * (g + 1)],
                        rhs=rhs0[cur][:, :],
                        start=True,
                        stop=True,
                    )
                # one sigmoid over all 4 gates: [i, f, o, 2g]
                s0 = SIGP.tile([128, 2048], BF16, tag="sig0", name="sig0")
                nc.scalar.activation(out=s0[:, :], in_=g0[:, :], func=SIG)
                # tanh(g) = 2*sigmoid(2g) - 1  (4x-mode tensor_scalar)
                tg0 = TGP.tile([96, BS], BF16, tag="tg0", name="tg0")
                nc.vector.tensor_scalar(
                    tg0[:, :], s0[0:96, 1536:2048], 2.0, 1.0, MULT, SUB
                )
                q0 = QPP.tile([96, BS], BF16, tag="q0", name="q0")
                p0 = QPP.tile([96, BS], BF16, tag="p0", name="p0")
                nc.vector.tensor_mul(q0[:, :], s0[0:96, 512:1024], c0[:, :])
                nc.vector.tensor_mul(p0[:, :], s0[0:96, 0:512], tg0[:, :])
                nc.vector.tensor_add(c0[:, :], q0[:, :], p0[:, :])
                tc0 = TCP.tile([96, BS], BF16, tag="tc0", name="tc0")
                nc.scalar.activation(out=tc0[:, :], in_=c0[:, :], func=TANH)
                nc.vector.tensor_mul(
                    rhs0[nxt][0:96, :], s0[0:96, 1024:1536], tc0[:, :]
                )

            def w1b_pre(t):
                # w1b*[h2;1] accumulation for L1 step t: only needs h2(t-1),
                # so it is emitted at the tail of iteration t-1 and executes
                # in the PE idle window while the PE waits for h1(t).
                cur = t % 2
                g1 = PSP.tile([128, 2048], F32, tag="g1", name="g1")
                for g in range(4):
                    nc.tensor.matmul(
                        out=g1[:, 512 * g : 512 * (g + 1)],
                        lhsT=w1b_s[:, 128 * g : 128 * (g + 1)],
                        rhs=rhs1[cur][0:97, :],
                        start=True,
                        stop=False,
                    )
                return g1

            def l1_rest(t, g1):
                # layer-1 step t: w1a*h1(t) accumulation, activations, cell
                # update; writes h2_t into rhs1[(t+1)%2]
                nxt = (t + 1) % 2
                for g in range(4):
                    nc.tensor.matmul(
                        out=g1[:, 512 * g : 512 * (g + 1)],
                        lhsT=w1a_s[:, 128 * g : 128 * (g + 1)],
                        rhs=rhs0[nxt][0:96, :],
                        start=False,
                        stop=True,
                    )
                s1 = SIGP.tile([128, 2048], BF16, tag="sig1", name="sig1")
                nc.scalar.activation(out=s1[:, :], in_=g1[:, :], func=SIG)
                tg1 = TGP.tile([96, BS], BF16, tag="tg1", name="tg1")
                nc.vector.tensor_scalar(
                    tg1[:, :], s1[0:96, 1536:2048], 2.0, 1.0, MULT, SUB
                )
                q1 = QPP.tile([96, BS], BF16, tag="q1", name="q1")
                p1 = QPP.tile([96, BS], BF16, tag="p1", name="p1")
                nc.vector.tensor_mul(q1[:, :], s1[0:96, 512:1024], c1[:, :])
                nc.vector.tensor_mul(p1[:, :], s1[0:96, 0:512], tg1[:, :])
                nc.vector.tensor_add(c1[:, :], q1[:, :], p1[:, :])
                tc1 = TCP.tile([96, BS], BF16, tag="tc1", name="tc1")
                nc.scalar.activation(out=tc1[:, :], in_=c1[:, :], func=TANH)
                nc.vector.tensor_mul(
                    rhs1[nxt][0:96, :], s1[0:96, 1024:1536], tc1[:, :]
                )

            # Software-pipelined emission. Per iteration t the per-engine
            # queue order is:
            #   PE:  L0 mms(t+1) -> w1a(t) -> w1b(t+1) [parks until h2(t)]
            #   ACT: sig0(t+1) -> tanh0(t+1) -> sig1(t) -> tanh1(t)
            #   DVE: L0 chain(t+1) -> h1-mul(t+1) -> L1 chain(t) -> h2-mul(t)
            # The 4-deep per-engine wait queues let a ready instruction pass
            # a parked one, so this order prioritizes the recurrence-critical
            # L0 spine while L1 work fills the gaps; w1b(t+1) executes in the
            # PE idle window while waiting for h1(t+1).
            g1 = w1b_pre(0)
            l0_block(0)
            for t in range(T_RUN):
                if t + 1 < T_RUN:
                    l0_block(t + 1)
                l1_rest(t, g1)
                if t + 1 < T_RUN:
                    g1 = w1b_pre(t + 1)

            # ---- FC head on h2 at t = T-1 ----
            fc_ps = PSP.tile([1, 512], F32, tag="g0")
            nc.tensor.matmul(
                out=fc_ps[:, :],
                lhsT=wfc_s[:, :],
                rhs=rhs1[T_RUN % 2][0:97, :],
                start=True,
                stop=True,
            )
            y_s = P.tile([1, 512], F32, tag="y")
            nc.vector.tensor_copy(y_s[:, :], fc_ps[:, :])
            nc.gpsimd.dma_start(out=y_d[:, :], in_=y_s[:, :])
    nc.compile()
    return nc



def _ensure_ntff_hook():
    """Provide antenv.axon_hooks (absent in this image) so trace=True works."""
    import sys, types, ctypes, contextlib
    try:
        import antenv.axon_hooks  # noqa: F401
        return
    except ImportError:
        pass
    mod = types.ModuleType("antenv.axon_hooks")
    holder = {}
    mod.set_axon_ntff_profile_hook = lambda h: holder.__setitem__("h", h)
    mod.get_axon_ntff_profile_hook = lambda: holder.get("h")
    sys.modules["antenv.axon_hooks"] = mod
    lib = ctypes.CDLL("/opt/axon/libaxon_pjrt.so")
    if not hasattr(lib, "axon_start_nrt_profile"):
        return
    lib.axon_start_nrt_profile.argtypes = [
        ctypes.POINTER(ctypes.c_int64), ctypes.c_size_t]
    lib.axon_start_nrt_profile.restype = ctypes.c_int64
    lib.axon_stop_nrt_profile.argtypes = [ctypes.c_char_p]
    lib.axon_stop_nrt_profile.restype = ctypes.c_int64

    @contextlib.contextmanager
    def _hook(output_dir, device_ids):
        import jax
        jax.devices()
        if device_ids:
            ids = (ctypes.c_int64 * len(device_ids))(*device_ids)
            rc = lib.axon_start_nrt_profile(ids, len(device_ids))
        else:
            rc = lib.axon_start_nrt_profile(None, 0)
        if rc != 0:
            raise RuntimeError(f"axon_start_nrt_profile rc={rc}")
        try:
            yield
        finally:
            n = lib.axon_stop_nrt_profile(str(output_dir).encode())
            print(f"ntff profile: {n} file(s) written to {output_dir}")

    mod.set_axon_ntff_profile_hook(_hook)


def _patch_upload():
    """Skip artifact upload to remote storage (no share in this container)."""
    import concourse.bass_utils as bu
    bu.upload_artifacts = lambda tmpdir: tmpdir


_NC = None


def kernel(x, Wih0, Whh0, bih0, bhh0, Wih1, Whh1, bih1, bhh1, Wfc, bfc):
    global _NC
    arrs = [np.asarray(a, np.float32) for a in (
        x, Wih0, Whh0, bih0, bhh0, Wih1, Whh1, bih1, bhh1, Wfc, bfc)]
    x = arrs[0]
    w0, w1a, w1b, wfc = _prep_weights(*arrs[1:])
    if _NC is None:
        _NC = _build_nc()
    bf = ml_dtypes.bfloat16
    in_maps = []
    for core in range(NCORES):
        xt = x[core * BS : (core + 1) * BS].transpose(1, 2, 0)  # [T, D, BS]
        xs = np.concatenate(
            [xt, np.ones((T, 1, BS), np.float32)], axis=1
        ).astype(bf)  # [T, D+1, BS] with const-1 row
        in_maps.append({"xs": xs, "w0": w0, "w1a": w1a, "w1b": w1b, "wfc": wfc})
    if TRACE:
        _ensure_ntff_hook()
        _patch_upload()
    import tempfile
    tdir = tempfile.mkdtemp(prefix="lstm_prof_") if TRACE else None
    res = run_bass_kernel_spmd(
        _NC, in_maps, core_ids=list(range(NCORES)), trace=TRACE, tmpdir=tdir
    )
    LAST["tmpdir"] = tdir
    LAST["exec_time_ns"] = res.exec_time_ns
    LAST["profile_json"] = res.profile_json
    y = np.concatenate([res.results[i]["y"][0] for i in range(NCORES)])
    return y.astype(np.float32)


# revision 7
# speedup vs baseline: 1.4197x; 1.1295x over previous
"""Trainium2 Bass kernel for a 2-layer LSTM (B=4096, T=168, D=16, H=96) + FC head.

Strategy: pure data parallel over 8 NeuronCores (512 batch rows each).
Per core, gate-major layout: the recurrent matmul computes gates.T
[gate, batch] with weights stationary on the PE, so hidden state h stays in
[feature, batch] layout across steps and never needs a transpose.

v2 changes vs v1 (1493us):
- bf16 matmuls (lhsT + rhs): 1 cycle/row at any N, halves weight-load time,
  halves DMA, and lets DVE run its 2x/4x perf modes on the elementwise ops.
- ONE sigmoid ACT op per layer-step covering all 4 gates [128, 2048]: gate g's
  weights+bias are pre-scaled by 2 so sigmoid(2g) comes out, and
  tanh(g) = 2*sigmoid(2g) - 1 is fixed up with a 4x-mode DVE tensor_scalar.
  This cuts ACT from 3 to 2 instructions per layer-step (ACT is the
  bottleneck engine: cost = free-size * 0.83ns, dtype-independent).
- tanh(c) stays a real ACT tanh (same instruction count either way).
- All elementwise tiles bf16 in SBUF: tensor_tensor runs 2x (2x_1p),
  tensor_scalar runs 4x (4x_2p).

Gates are ordered [i, f, o, g], each padded to 128 PSUM partitions. Biases
ride along in the matmul via a constant-1.0 input row.
"""

import numpy as np
import ml_dtypes

import concourse.bass as bass
import concourse.bacc as bacc
import concourse.tile as tile
from concourse import mybir
from concourse.bass_utils import run_bass_kernel_spmd

B, T, D, H = 4096, 168, 16, 96
NCORES = 8
BS = B // NCORES  # 512 batch rows per core
F32 = mybir.dt.float32
BF16 = mybir.dt.bfloat16
SIG = mybir.ActivationFunctionType.Sigmoid
TANH = mybir.ActivationFunctionType.Tanh
MULT = mybir.AluOpType.mult
SUB = mybir.AluOpType.subtract

# gate row slices in torch order (i, f, g, o) -> our tile order [i, f, o, g]
_GATE_SLICES = [(0, 96), (96, 192), (288, 384), (192, 288)]

TRACE = False
LAST = {}
T_RUN = T
MM_DT = BF16
# PE idle-filler matmuls per iteration: keeps the PE streaming continuously so
# it ramps to (and holds) its 2.4GHz p-state instead of the 1.2GHz mid state
# it falls back to after any idle gap. Filler output is garbage written to a
# PSUM region that the next real start=True matmul fully overwrites.
JUNK_MM = 6
JUNK_N = 512


def _prep_weights(Wih0, Whh0, bih0, bhh0, Wih1, Whh1, bih1, bhh1, Wfc, bfc):
    w0 = np.zeros((113, 512), np.float32)  # rows: h(96), x(16), const(1)
    w1a = np.zeros((96, 512), np.float32)  # rows: h1(96)
    w1b = np.zeros((97, 512), np.float32)  # rows: h2(96), const(1)
    for gi, (r0, r1) in enumerate(_GATE_SLICES):
        c0, c1 = 128 * gi, 128 * gi + 96
        # gate 3 is g: pre-scale by 2 so the merged sigmoid computes
        # sigmoid(2g) and tanh(g) = 2*sigmoid(2g) - 1
        sc = 2.0 if gi == 3 else 1.0
        w0[0:96, c0:c1] = sc * Whh0[r0:r1, :].T
        w0[96:112, c0:c1] = sc * Wih0[r0:r1, :].T
        w0[112, c0:c1] = sc * (bih0[r0:r1] + bhh0[r0:r1])
        w1a[:, c0:c1] = sc * Wih1[r0:r1, :].T
        w1b[0:96, c0:c1] = sc * Whh1[r0:r1, :].T
        w1b[96, c0:c1] = sc * (bih1[r0:r1] + bhh1[r0:r1])
    wfc = np.zeros((97, 1), np.float32)
    wfc[0:96, 0] = Wfc[0, :]
    wfc[96, 0] = bfc[0]
    bf = ml_dtypes.bfloat16
    return w0.astype(bf), w1a.astype(bf), w1b.astype(bf), wfc.astype(bf)


def _build_nc():
    nc = bacc.Bacc("TRN2", target_bir_lowering=False)
    xs_d = nc.dram_tensor("xs", [T, D + 1, BS], MM_DT, kind="ExternalInput")
    w0_d = nc.dram_tensor("w0", [113, 512], MM_DT, kind="ExternalInput")
    w1a_d = nc.dram_tensor("w1a", [96, 512], MM_DT, kind="ExternalInput")
    w1b_d = nc.dram_tensor("w1b", [97, 512], MM_DT, kind="ExternalInput")
    wfc_d = nc.dram_tensor("wfc", [97, 1], MM_DT, kind="ExternalInput")
    y_d = nc.dram_tensor("y", [1, BS], F32, kind="ExternalOutput")

    with tile.TileContext(nc) as tc:
        with (
            tc.tile_pool(name="persist", bufs=1) as P,
            tc.tile_pool(name="sig", bufs=2) as SIGP,
            tc.tile_pool(name="tgp", bufs=2) as TGP,
            tc.tile_pool(name="tcp", bufs=2) as TCP,
            tc.tile_pool(name="qp", bufs=2) as QPP,
            tc.tile_pool(name="ps", bufs=1, space="PSUM") as PSP,
        ):
            # DMA into staging tiles, then DVE-copy into the tiles matmuls
            # read, so matmul waits only involve {DVE, ACT} sems.
            w0_g = P.tile([113, 512], MM_DT, tag="w0_g")
            w1a_g = P.tile([96, 512], MM_DT, tag="w1a_g")
            w1b_g = P.tile([97, 512], MM_DT, tag="w1b_g")
            wfc_g = P.tile([97, 1], MM_DT, tag="wfc_g")
            nc.gpsimd.dma_start(out=w0_g[:, :], in_=w0_d[:, :])
            nc.gpsimd.dma_start(out=w1a_g[:, :], in_=w1a_d[:, :])
            nc.gpsimd.dma_start(out=w1b_g[:, :], in_=w1b_d[:, :])
            nc.gpsimd.dma_start(out=wfc_g[:, :], in_=wfc_d[:, :])
            w0_s = P.tile([113, 512], MM_DT, tag="w0")
            w1a_s = P.tile([96, 512], MM_DT, tag="w1a")
            w1b_s = P.tile([97, 512], MM_DT, tag="w1b")
            wfc_s = P.tile([97, 1], MM_DT, tag="wfc")
            nc.vector.tensor_copy(w0_s[:, :], w0_g[:, :])
            nc.vector.tensor_copy(w1a_s[:, :], w1a_g[:, :])
            nc.vector.tensor_copy(w1b_s[:, :], w1b_g[:, :])
            nc.vector.tensor_copy(wfc_s[:, :], wfc_g[:, :])

            # rhs0: [h0(0:96); x_t(96:112); 1.0(112)]  rhs1: [h2(0:96); 1.0(96)]
            rhs0 = [P.tile([113, BS], MM_DT, tag=f"rhs0_{i}", name=f"rhs0_{i}") for i in range(2)]
            rhs1 = [P.tile([97, BS], MM_DT, tag=f"rhs1_{i}", name=f"rhs1_{i}") for i in range(2)]
            c0 = P.tile([96, BS], BF16, tag="c0")
            c1 = P.tile([96, BS], BF16, tag="c1")
            for i in range(2):
                nc.vector.memset(rhs0[i][:, :], 0.0)
                nc.vector.memset(rhs1[i][:, :], 0.0)
                nc.vector.memset(rhs1[i][96:97, :], 1.0)
            nc.vector.memset(c0[:, :], 0.0)
            nc.vector.memset(c1[:, :], 0.0)

            nc.gpsimd.dma_start(out=rhs0[0][96:113, :], in_=xs_d[0, :, :])

            def l0_block(t):
                # layer-0 step t: consumes rhs0[t%2], writes h1_t into
                # rhs0[(t+1)%2] rows 0:96
                cur, nxt = t % 2, (t + 1) % 2
                if t + 1 < T_RUN:
                    nc.gpsimd.dma_start(
                        out=rhs0[nxt][96:113, :], in_=xs_d[t + 1, :, :]
                    )
                g0 = PSP.tile([128, 2048], F32, tag="g0", name="g0")
                for g in range(4):
                    nc.tensor.matmul(
                        out=g0[:, 512 * g : 512 * (g + 1)],
                        lhsT=w0_s[:, 128 * g : 128 * (g + 1)],
                        rhs=rhs0[cur][:, :],
                        start=True,
                        stop=True,
                    )
                # one sigmoid over all 4 gates: [i, f, o, 2g]
                s0 = SIGP.tile([128, 2048], BF16, tag="sig0", name="sig0")
                nc.scalar.activation(out=s0[:, :], in_=g0[:, :], func=SIG)
                # tanh(g) = 2*sigmoid(2g) - 1  (4x-mode tensor_scalar)
                tg0 = TGP.tile([96, BS], BF16, tag="tg0", name="tg0")
                nc.vector.tensor_scalar(
                    tg0[:, :], s0[0:96, 1536:2048], 2.0, 1.0, MULT, SUB
                )
                q0 = QPP.tile([96, BS], BF16, tag="q0", name="q0")
                p0 = QPP.tile([96, BS], BF16, tag="p0", name="p0")
                nc.vector.tensor_mul(q0[:, :], s0[0:96, 512:1024], c0[:, :])
                nc.vector.tensor_mul(p0[:, :], s0[0:96, 0:512], tg0[:, :])
                nc.vector.tensor_add(c0[:, :], q0[:, :], p0[:, :])
                tc0 = TCP.tile([96, BS], BF16, tag="tc0", name="tc0")
                nc.scalar.activation(out=tc0[:, :], in_=c0[:, :], func=TANH)
                nc.vector.tensor_mul(
                    rhs0[nxt][0:96, :], s0[0:96, 1024:1536], tc0[:, :]
                )

            def l1_block(t):
                # layer-1 step t: consumes h1_t (rhs0[(t+1)%2]) and rhs1[t%2],
                # writes h2_t into rhs1[(t+1)%2]
                cur, nxt = t % 2, (t + 1) % 2
                g1 = PSP.tile([128, 2048], F32, tag="g1", name="g1")
                for g in range(4):
                    nc.tensor.matmul(
                        out=g1[:, 512 * g : 512 * (g + 1)],
                        lhsT=w1a_s[:, 128 * g : 128 * (g + 1)],
                        rhs=rhs0[nxt][0:96, :],
                        start=True,
                        stop=False,
                    )
                    nc.tensor.matmul(
                        out=g1[:, 512 * g : 512 * (g + 1)],
                        lhsT=w1b_s[:, 128 * g : 128 * (g + 1)],
                        rhs=rhs1[cur][0:97, :],
                        start=False,
                        stop=True,
                    )
                s1 = SIGP.tile([128, 2048], BF16, tag="sig1", name="sig1")
                # sigma1 split in two: its ACT occupancy blocks the spine's
                # tanh0 (which becomes ready mid-sigma1); halving the
                # granularity halves the expected blocking time.
                nc.scalar.activation(
                    out=s1[:, 0:1024], in_=g1[:, 0:1024], func=SIG
                )
                nc.scalar.activation(
                    out=s1[:, 1024:2048], in_=g1[:, 1024:2048], func=SIG
                )
                tg1 = TGP.tile([96, BS], BF16, tag="tg1", name="tg1")
                nc.vector.tensor_scalar(
                    tg1[:, :], s1[0:96, 1536:2048], 2.0, 1.0, MULT, SUB
                )
                q1 = QPP.tile([96, BS], BF16, tag="q1", name="q1")
                p1 = QPP.tile([96, BS], BF16, tag="p1", name="p1")
                nc.vector.tensor_mul(q1[:, :], s1[0:96, 512:1024], c1[:, :])
                nc.vector.tensor_mul(p1[:, :], s1[0:96, 0:512], tg1[:, :])
                nc.vector.tensor_add(c1[:, :], q1[:, :], p1[:, :])
                tc1 = TCP.tile([96, BS], BF16, tag="tc1", name="tc1")
                nc.scalar.activation(out=tc1[:, :], in_=c1[:, :], func=TANH)
                nc.vector.tensor_mul(
                    rhs1[nxt][0:96, :], s1[0:96, 1024:1536], tc1[:, :]
                )

            # Software-pipelined emission: the L0 chain for step t+1 is
            # emitted BEFORE the L1 chain for step t, so the PE FIFO (and
            # ACT/DVE queues) prioritize the recurrence-critical L0 loop
            # while L1 work fills the gaps.
            l0_block(0)
            for t in range(T_RUN):
                if t + 1 < T_RUN:
                    l0_block(t + 1)
                l1_block(t)

            # ---- FC head on h2 at t = T-1 ----
            fc_ps = PSP.tile([1, 512], F32, tag="g0")
            nc.tensor.matmul(
                out=fc_ps[:, :],
                lhsT=wfc_s[:, :],
                rhs=rhs1[T_RUN % 2][0:97, :],
                start=True,
                stop=True,
            )
            y_s = P.tile([1, 512], F32, tag="y")
            nc.vector.tensor_copy(y_s[:, :], fc_ps[:, :])
            nc.gpsimd.dma_start(out=y_d[:, :], in_=y_s[:, :])
    nc.compile()
    return nc



def _ensure_ntff_hook():
    """Provide antenv.axon_hooks (absent in this image) so trace=True works."""
    import sys, types, ctypes, contextlib
    try:
        import antenv.axon_hooks  # noqa: F401
        return
    except ImportError:
        pass
    mod = types.ModuleType("antenv.axon_hooks")
    holder = {}
    mod.set_axon_ntff_profile_hook = lambda h: holder.__setitem__("h", h)
    mod.get_axon_ntff_profile_hook = lambda: holder.get("h")
    sys.modules["antenv.axon_hooks"] = mod
    lib = ctypes.CDLL("/opt/axon/libaxon_pjrt.so")
    if not hasattr(lib, "axon_start_nrt_profile"):
        return
    lib.axon_start_nrt_profile.argtypes = [
        ctypes.POINTER(ctypes.c_int64), ctypes.c_size_t]
    lib.axon_start_nrt_profile.restype = ctypes.c_int64
    lib.axon_stop_nrt_profile.argtypes = [ctypes.c_char_p]
    lib.axon_stop_nrt_profile.restype = ctypes.c_int64

    @contextlib.contextmanager
    def _hook(output_dir, device_ids):
        import jax
        jax.devices()
        if device_ids:
            ids = (ctypes.c_int64 * len(device_ids))(*device_ids)
            rc = lib.axon_start_nrt_profile(ids, len(device_ids))
        else:
            rc = lib.axon_start_nrt_profile(None, 0)
        if rc != 0:
            raise RuntimeError(f"axon_start_nrt_profile rc={rc}")
        try:
            yield
        finally:
            n = lib.axon_stop_nrt_profile(str(output_dir).encode())
            print(f"ntff profile: {n} file(s) written to {output_dir}")

    mod.set_axon_ntff_profile_hook(_hook)


def _patch_upload():
    """Skip artifact upload to remote storage (no share in this container)."""
    import concourse.bass_utils as bu
    bu.upload_artifacts = lambda tmpdir: tmpdir


_NC = None


def kernel(x, Wih0, Whh0, bih0, bhh0, Wih1, Whh1, bih1, bhh1, Wfc, bfc):
    global _NC
    arrs = [np.asarray(a, np.float32) for a in (
        x, Wih0, Whh0, bih0, bhh0, Wih1, Whh1, bih1, bhh1, Wfc, bfc)]
    x = arrs[0]
    w0, w1a, w1b, wfc = _prep_weights(*arrs[1:])
    if _NC is None:
        _NC = _build_nc()
    bf = ml_dtypes.bfloat16
    in_maps = []
    for core in range(NCORES):
        xt = x[core * BS : (core + 1) * BS].transpose(1, 2, 0)  # [T, D, BS]
        xs = np.concatenate(
            [xt, np.ones((T, 1, BS), np.float32)], axis=1
        ).astype(bf)  # [T, D+1, BS] with const-1 row
        in_maps.append({"xs": xs, "w0": w0, "w1a": w1a, "w1b": w1b, "wfc": wfc})
    if TRACE:
        _ensure_ntff_hook()
        _patch_upload()
    import tempfile
    tdir = tempfile.mkdtemp(prefix="lstm_prof_") if TRACE else None
    res = run_bass_kernel_spmd(
        _NC, in_maps, core_ids=list(range(NCORES)), trace=TRACE, tmpdir=tdir
    )
    LAST["tmpdir"] = tdir
    LAST["exec_time_ns"] = res.exec_time_ns
    LAST["profile_json"] = res.profile_json
    y = np.concatenate([res.results[i]["y"][0] for i in range(NCORES)])
    return y.astype(np.float32)


# revision 9
# speedup vs baseline: 1.6288x; 1.1473x over previous
"""Trainium2 Bass kernel for a 2-layer LSTM (B=4096, T=168, D=16, H=96) + FC head.

Strategy: pure data parallel over 8 NeuronCores (512 batch rows each).
Per core, gate-major layout: the recurrent matmul computes gates.T
[gate, batch] with weights stationary on the PE, so hidden state h stays in
[feature, batch] layout across steps and never needs a transpose.

v2 changes vs v1 (1493us):
- bf16 matmuls (lhsT + rhs): 1 cycle/row at any N, halves weight-load time,
  halves DMA, and lets DVE run its 2x/4x perf modes on the elementwise ops.
- ONE sigmoid ACT op per layer-step covering all 4 gates [128, 2048]: gate g's
  weights+bias are pre-scaled by 2 so sigmoid(2g) comes out, and
  tanh(g) = 2*sigmoid(2g) - 1 is fixed up with a 4x-mode DVE tensor_scalar.
  This cuts ACT from 3 to 2 instructions per layer-step (ACT is the
  bottleneck engine: cost = free-size * 0.83ns, dtype-independent).
- tanh(c) stays a real ACT tanh (same instruction count either way).
- All elementwise tiles bf16 in SBUF: tensor_tensor runs 2x (2x_1p),
  tensor_scalar runs 4x (4x_2p).

Gates are ordered [i, f, o, g], each padded to 128 PSUM partitions. Biases
ride along in the matmul via a constant-1.0 input row.
"""

import numpy as np
import ml_dtypes

import concourse.bass as bass
import concourse.bacc as bacc
import concourse.tile as tile
from concourse import mybir
from concourse.bass_utils import run_bass_kernel_spmd

B, T, D, H = 4096, 168, 16, 96
NCORES = 8
BS = B // NCORES  # 512 batch rows per core
F32 = mybir.dt.float32
BF16 = mybir.dt.bfloat16
SIG = mybir.ActivationFunctionType.Sigmoid
TANH = mybir.ActivationFunctionType.Tanh
MULT = mybir.AluOpType.mult
SUB = mybir.AluOpType.subtract

# gate row slices in torch order (i, f, g, o) -> our tile order [i, f, o, g]
_GATE_SLICES = [(0, 96), (96, 192), (288, 384), (192, 288)]

TRACE = False
LAST = {}
T_RUN = T
MM_DT = BF16
# One garbage matmul between L1's psA and psB matmul groups (output is
# overwritten by the next start=True matmul). Delays sigma1b's readiness so
# the spine's tanh0 wins the ACT engine.
JUNK_MM = 1
JUNK_N = 512


def _prep_weights(Wih0, Whh0, bih0, bhh0, Wih1, Whh1, bih1, bhh1, Wfc, bfc):
    w0 = np.zeros((113, 512), np.float32)  # rows: h(96), x(16), const(1)
    w1a = np.zeros((96, 512), np.float32)  # rows: h1(96)
    w1b = np.zeros((97, 512), np.float32)  # rows: h2(96), const(1)
    for gi, (r0, r1) in enumerate(_GATE_SLICES):
        c0, c1 = 128 * gi, 128 * gi + 96
        # gate 3 is g: pre-scale by 2 so the merged sigmoid computes
        # sigmoid(2g) and tanh(g) = 2*sigmoid(2g) - 1
        sc = 2.0 if gi == 3 else 1.0
        w0[0:96, c0:c1] = sc * Whh0[r0:r1, :].T
        w0[96:112, c0:c1] = sc * Wih0[r0:r1, :].T
        w0[112, c0:c1] = sc * (bih0[r0:r1] + bhh0[r0:r1])
        w1a[:, c0:c1] = sc * Wih1[r0:r1, :].T
        w1b[0:96, c0:c1] = sc * Whh1[r0:r1, :].T
        w1b[96, c0:c1] = sc * (bih1[r0:r1] + bhh1[r0:r1])
    wfc = np.zeros((97, 1), np.float32)
    wfc[0:96, 0] = Wfc[0, :]
    wfc[96, 0] = bfc[0]
    bf = ml_dtypes.bfloat16
    return w0.astype(bf), w1a.astype(bf), w1b.astype(bf), wfc.astype(bf)


def _build_nc():
    nc = bacc.Bacc("TRN2", target_bir_lowering=False)
    xs_d = nc.dram_tensor("xs", [T, D + 1, BS], MM_DT, kind="ExternalInput")
    w0_d = nc.dram_tensor("w0", [113, 512], MM_DT, kind="ExternalInput")
    w1a_d = nc.dram_tensor("w1a", [96, 512], MM_DT, kind="ExternalInput")
    w1b_d = nc.dram_tensor("w1b", [97, 512], MM_DT, kind="ExternalInput")
    wfc_d = nc.dram_tensor("wfc", [97, 1], MM_DT, kind="ExternalInput")
    y_d = nc.dram_tensor("y", [1, BS], F32, kind="ExternalOutput")

    with tile.TileContext(nc) as tc:
        with (
            tc.tile_pool(name="persist", bufs=1) as P,
            tc.tile_pool(name="sig", bufs=2) as SIGP,
            tc.tile_pool(name="tgp", bufs=2) as TGP,
            tc.tile_pool(name="tcp", bufs=2) as TCP,
            tc.tile_pool(name="qp", bufs=2) as QPP,
            tc.tile_pool(name="ps", bufs=1, space="PSUM") as PSP,
        ):
            # DMA into staging tiles, then DVE-copy into the tiles matmuls
            # read, so matmul waits only involve {DVE, ACT} sems.
            w0_g = P.tile([113, 512], MM_DT, tag="w0_g")
            w1a_g = P.tile([96, 512], MM_DT, tag="w1a_g")
            w1b_g = P.tile([97, 512], MM_DT, tag="w1b_g")
            wfc_g = P.tile([97, 1], MM_DT, tag="wfc_g")
            nc.gpsimd.dma_start(out=w0_g[:, :], in_=w0_d[:, :])
            nc.gpsimd.dma_start(out=w1a_g[:, :], in_=w1a_d[:, :])
            nc.gpsimd.dma_start(out=w1b_g[:, :], in_=w1b_d[:, :])
            nc.gpsimd.dma_start(out=wfc_g[:, :], in_=wfc_d[:, :])
            w0_s = P.tile([113, 512], MM_DT, tag="w0")
            w1a_s = P.tile([96, 512], MM_DT, tag="w1a")
            w1b_s = P.tile([97, 512], MM_DT, tag="w1b")
            wfc_s = P.tile([97, 1], MM_DT, tag="wfc")
            nc.vector.tensor_copy(w0_s[:, :], w0_g[:, :])
            nc.vector.tensor_copy(w1a_s[:, :], w1a_g[:, :])
            nc.vector.tensor_copy(w1b_s[:, :], w1b_g[:, :])
            nc.vector.tensor_copy(wfc_s[:, :], wfc_g[:, :])

            # rhs0: [h0(0:96); x_t(96:112); 1.0(112)]  rhs1: [h2(0:96); 1.0(96)]
            rhs0 = [P.tile([113, BS], MM_DT, tag=f"rhs0_{i}", name=f"rhs0_{i}") for i in range(2)]
            rhs1 = [P.tile([97, BS], MM_DT, tag=f"rhs1_{i}", name=f"rhs1_{i}") for i in range(2)]
            c0 = P.tile([96, BS], BF16, tag="c0")
            c1 = P.tile([96, BS], BF16, tag="c1")
            for i in range(2):
                nc.vector.memset(rhs0[i][:, :], 0.0)
                nc.vector.memset(rhs1[i][:, :], 0.0)
                nc.vector.memset(rhs1[i][96:97, :], 1.0)
            nc.vector.memset(c0[:, :], 0.0)
            nc.vector.memset(c1[:, :], 0.0)

            nc.gpsimd.dma_start(out=rhs0[0][96:113, :], in_=xs_d[0, :, :])

            def l0_block(t):
                # layer-0 step t: consumes rhs0[t%2], writes h1_t into
                # rhs0[(t+1)%2] rows 0:96
                cur, nxt = t % 2, (t + 1) % 2
                if t + 1 < T_RUN:
                    nc.gpsimd.dma_start(
                        out=rhs0[nxt][96:113, :], in_=xs_d[t + 1, :, :]
                    )
                g0 = PSP.tile([128, 2048], F32, tag="g0", name="g0")
                for g in range(4):
                    nc.tensor.matmul(
                        out=g0[:, 512 * g : 512 * (g + 1)],
                        lhsT=w0_s[:, 128 * g : 128 * (g + 1)],
                        rhs=rhs0[cur][:, :],
                        start=True,
                        stop=True,
                    )
                # one sigmoid over all 4 gates: [i, f, o, 2g]
                s0 = SIGP.tile([128, 2048], BF16, tag="sig0", name="sig0")
                nc.scalar.activation(out=s0[:, :], in_=g0[:, :], func=SIG)
                # tanh(g) = 2*sigmoid(2g) - 1  (4x-mode tensor_scalar)
                tg0 = TGP.tile([96, BS], BF16, tag="tg0", name="tg0")
                nc.vector.tensor_scalar(
                    tg0[:, :], s0[0:96, 1536:2048], 2.0, 1.0, MULT, SUB
                )
                q0 = QPP.tile([96, BS], BF16, tag="q0", name="q0")
                p0 = QPP.tile([96, BS], BF16, tag="p0", name="p0")
                nc.vector.tensor_mul(q0[:, :], s0[0:96, 512:1024], c0[:, :])
                nc.vector.tensor_mul(p0[:, :], s0[0:96, 0:512], tg0[:, :])
                nc.vector.tensor_add(c0[:, :], q0[:, :], p0[:, :])
                tc0 = TCP.tile([96, BS], BF16, tag="tc0", name="tc0")
                nc.scalar.activation(out=tc0[:, :], in_=c0[:, :], func=TANH)
                nc.vector.tensor_mul(
                    rhs0[nxt][0:96, :], s0[0:96, 1024:1536], tc0[:, :]
                )

            def l1_block(t):
                # layer-1 step t: consumes h1_t (rhs0[(t+1)%2]) and rhs1[t%2],
                # writes h2_t into rhs1[(t+1)%2].
                # L1's PSUM is split into two 2-bank tiles (gates i,f -> psA;
                # o,g -> psB) with sigma1 split accordingly: each half's
                # matmuls for step t+1 then only wait on that half's sigma1
                # read for step t, which unwinds the L1 lag cascade a single
                # 4-bank tile causes. sigma1a is emitted right after the A
                # matmuls so it fills the ACT gap behind sigma0.
                cur, nxt = t % 2, (t + 1) % 2
                psA = PSP.tile([128, 1024], F32, tag="g1a", name="g1a")
                psB = PSP.tile([128, 1024], F32, tag="g1b", name="g1b")
                s1 = SIGP.tile([128, 2048], BF16, tag="sig1", name="sig1")
                for g in range(2):
                    nc.tensor.matmul(
                        out=psA[:, 512 * g : 512 * (g + 1)],
                        lhsT=w1a_s[:, 128 * g : 128 * (g + 1)],
                        rhs=rhs0[nxt][0:96, :],
                        start=True,
                        stop=False,
                    )
                    nc.tensor.matmul(
                        out=psA[:, 512 * g : 512 * (g + 1)],
                        lhsT=w1b_s[:, 128 * g : 128 * (g + 1)],
                        rhs=rhs1[cur][0:97, :],
                        start=False,
                        stop=True,
                    )
                nc.scalar.activation(out=s1[:, 0:1024], in_=psA[:, :], func=SIG)
                if JUNK_MM:
                    # one garbage matmul into psA (overwritten by the next
                    # start=True) delays psB's completion just enough that
                    # tanh0(t+1) beats sigma1b to the ACT engine.
                    nc.tensor.matmul(
                        out=psA[:, 0:JUNK_N],
                        lhsT=w1b_s[:, 0:128],
                        rhs=rhs1[cur][0:97, 0:JUNK_N],
                        start=True,
                        stop=True,
                        skip_group_check=True,
                    )
                for g in range(2, 4):
                    nc.tensor.matmul(
                        out=psB[:, 512 * (g - 2) : 512 * (g - 1)],
                        lhsT=w1a_s[:, 128 * g : 128 * (g + 1)],
                        rhs=rhs0[nxt][0:96, :],
                        start=True,
                        stop=False,
                    )
                    nc.tensor.matmul(
                        out=psB[:, 512 * (g - 2) : 512 * (g - 1)],
                        lhsT=w1b_s[:, 128 * g : 128 * (g + 1)],
                        rhs=rhs1[cur][0:97, :],
                        start=False,
                        stop=True,
                    )
                nc.scalar.activation(
                    out=s1[:, 1024:2048], in_=psB[:, :], func=SIG
                )
                tg1 = TGP.tile([96, BS], BF16, tag="tg1", name="tg1")
                nc.vector.tensor_scalar(
                    tg1[:, :], s1[0:96, 1536:2048], 2.0, 1.0, MULT, SUB
                )
                q1 = QPP.tile([96, BS], BF16, tag="q1", name="q1")
                p1 = QPP.tile([96, BS], BF16, tag="p1", name="p1")
                nc.vector.tensor_mul(q1[:, :], s1[0:96, 512:1024], c1[:, :])
                nc.vector.tensor_mul(p1[:, :], s1[0:96, 0:512], tg1[:, :])
                nc.vector.tensor_add(c1[:, :], q1[:, :], p1[:, :])
                tc1 = TCP.tile([96, BS], BF16, tag="tc1", name="tc1")
                nc.scalar.activation(out=tc1[:, :], in_=c1[:, :], func=TANH)
                nc.vector.tensor_mul(
                    rhs1[nxt][0:96, :], s1[0:96, 1024:1536], tc1[:, :]
                )

            # Software-pipelined emission: the L0 chain for step t+1 is
            # emitted BEFORE the L1 chain for step t, so the PE FIFO (and
            # ACT/DVE queues) prioritize the recurrence-critical L0 loop
            # while L1 work fills the gaps.
            l0_block(0)
            for t in range(T_RUN):
                if t + 1 < T_RUN:
                    l0_block(t + 1)
                l1_block(t)

            # ---- FC head on h2 at t = T-1 ----
            fc_ps = PSP.tile([1, 512], F32, tag="g0")
            nc.tensor.matmul(
                out=fc_ps[:, :],
                lhsT=wfc_s[:, :],
                rhs=rhs1[T_RUN % 2][0:97, :],
                start=True,
                stop=True,
            )
            y_s = P.tile([1, 512], F32, tag="y")
            nc.vector.tensor_copy(y_s[:, :], fc_ps[:, :])
            nc.gpsimd.dma_start(out=y_d[:, :], in_=y_s[:, :])
    nc.compile()
    return nc



def _ensure_ntff_hook():
    """Provide antenv.axon_hooks (absent in this image) so trace=True works."""
    import sys, types, ctypes, contextlib
    try:
        import antenv.axon_hooks  # noqa: F401
        return
    except ImportError:
        pass
    mod = types.ModuleType("antenv.axon_hooks")
    holder = {}
    mod.set_axon_ntff_profile_hook = lambda h: holder.__setitem__("h", h)
    mod.get_axon_ntff_profile_hook = lambda: holder.get("h")
    sys.modules["antenv.axon_hooks"] = mod
    lib = ctypes.CDLL("/opt/axon/libaxon_pjrt.so")
    if not hasattr(lib, "axon_start_nrt_profile"):
        return
    lib.axon_start_nrt_profile.argtypes = [
        ctypes.POINTER(ctypes.c_int64), ctypes.c_size_t]
    lib.axon_start_nrt_profile.restype = ctypes.c_int64
    lib.axon_stop_nrt_profile.argtypes = [ctypes.c_char_p]
    lib.axon_stop_nrt_profile.restype = ctypes.c_int64

    @contextlib.contextmanager
    def _hook(output_dir, device_ids):
        import jax
        jax.devices()
        if device_ids:
            ids = (ctypes.c_int64 * len(device_ids))(*device_ids)
            rc = lib.axon_start_nrt_profile(ids, len(device_ids))
        else:
            rc = lib.axon_start_nrt_profile(None, 0)
        if rc != 0:
            raise RuntimeError(f"axon_start_nrt_profile rc={rc}")
        try:
            yield
        finally:
            n = lib.axon_stop_nrt_profile(str(output_dir).encode())
            print(f"ntff profile: {n} file(s) written to {output_dir}")

    mod.set_axon_ntff_profile_hook(_hook)


def _patch_upload():
    """Skip artifact upload to remote storage (no share in this container)."""
    import concourse.bass_utils as bu
    bu.upload_artifacts = lambda tmpdir: tmpdir


_NC = None


def kernel(x, Wih0, Whh0, bih0, bhh0, Wih1, Whh1, bih1, bhh1, Wfc, bfc):
    global _NC
    arrs = [np.asarray(a, np.float32) for a in (
        x, Wih0, Whh0, bih0, bhh0, Wih1, Whh1, bih1, bhh1, Wfc, bfc)]
    x = arrs[0]
    w0, w1a, w1b, wfc = _prep_weights(*arrs[1:])
    if _NC is None:
        _NC = _build_nc()
    bf = ml_dtypes.bfloat16
    in_maps = []
    for core in range(NCORES):
        xt = x[core * BS : (core + 1) * BS].transpose(1, 2, 0)  # [T, D, BS]
        xs = np.concatenate(
            [xt, np.ones((T, 1, BS), np.float32)], axis=1
        ).astype(bf)  # [T, D+1, BS] with const-1 row
        in_maps.append({"xs": xs, "w0": w0, "w1a": w1a, "w1b": w1b, "wfc": wfc})
    if TRACE:
        _ensure_ntff_hook()
        _patch_upload()
    import tempfile
    tdir = tempfile.mkdtemp(prefix="lstm_prof_") if TRACE else None
    res = run_bass_kernel_spmd(
        _NC, in_maps, core_ids=list(range(NCORES)), trace=TRACE, tmpdir=tdir
    )
    LAST["tmpdir"] = tdir
    LAST["exec_time_ns"] = res.exec_time_ns
    LAST["profile_json"] = res.profile_json
    y = np.concatenate([res.results[i]["y"][0] for i in range(NCORES)])
    return y.astype(np.float32)
